# revision 37
# baseline (speedup 1.0000x reference)
"""De-stationary attention on 8 Trainium2 NeuronCores — ACT-bound pipeline.

Problem: y = softmax((x Wq^T + bq)(x Wk^T + bk)^T * scale / (tau*x_std)) (x Wv^T + bv) Wo^T + bo
Shapes: x [4, 2048, 1024], 16 heads of 64 dims, tau=1, delta=0.

Sharding: core c handles batch b = c//2, head group g = c%2 (8 heads).
s = SCALE/x_std[b] is folded into Wq/bq on the host. Host sums the two
head-group partial y's per batch and adds bo + bv @ Wo^T.

Core schedule (the scalar engine's exp stream is the bottleneck at
~1113 ns per [128,1024] tile x 256 = 285 us; everything else hides
under it):
  - x, Wq/Wk/Wv/Wo are bf16 (rel err ~7e-3, budget 2e-2); qT/kT/S
    stay fp32(r).
  - S^T tiles [128 keys, 512 q] computed per head-pair with K=64
    row-tiled matmuls (tile_position (0,0)/(64,0) auto-derived from
    base partitions) -> both heads' S matmuls run concurrently.
  - exp: one ACTIVATE per tk over [128, 1024] psum (both heads),
    double-buffered; output P in bf16.
  - PV: per head, stationary [v | ones] [128, 65] bf16 -> psum
    [65, 512] accumulated over 16 tk; row 64 = softmax denominator l.
  - Normalize: stage O+l out of psum fast, then l -> DRAM -> [64,16]
    -> DVE reciprocal -> DRAM -> broadcast-read -> multiply (bf16).
  - Blocks j-outer (head pair), tq-block rotated b = (bb+j)%4 so the
    output projection for block b unlocks early; projections for
    pairs 1-3, V projection, and output-projection chunks are fed as
    PE "filler" inside the exp-paced loop. Each block's tail PVs and
    normalization are deferred into the next block so the next S pair
    (which feeds the exp stream) issues first.
Emission order = dependency order: Tile resolves deps at creation
time, so a producer emitted after its consumer becomes a WAR hazard.
PSUM: ps_s 2x[128,1024] (4 banks) + ps_o 2x[65,512] (2) + filler
2x[128,512] (2) = 8 banks.
"""

import os
import sys

for _p in ("/opt/trn_rl_repo", "/root/.axon_site/_ro/trn_rl_repo"):
    if os.path.isdir(_p) and _p not in sys.path:
        sys.path.insert(0, _p)

import numpy as np
import ml_dtypes

import concourse.bass as bass
import concourse.mybir as mybir
import concourse.tile as tile
from concourse import bacc
from concourse.bass_utils import run_bass_kernel_spmd

F32 = mybir.dt.float32
F32R = mybir.dt.float32r
BF16 = mybir.dt.bfloat16
AF = mybir.ActivationFunctionType

B, T, D, H = 4, 2048, 1024, 16
HD = D // H          # 64
SCALE = HD ** -0.5
HG = H // 2          # 8 heads per core
EG = HG * HD         # 512 projection dims per core
N_CORES = 8

NTK = T // 128       # 16 key tiles
NB = T // 512        # 4 query blocks of 512
NJ = HG // 2         # 4 head pairs per core
VS = HD + 2          # 66: v cols + ones col + pad (keeps bf16 4B align)


def _build():
    nc = bacc.Bacc("TRN2", target_bir_lowering=False, debug=False)

    # x is host-packed to [128, NB*8*512]: quarter-major, then k-chunk, so
    # each query-quarter is ONE contiguous 1MB DMA with 8KB rows (chunked
    # 1KB-row DMAs only reach ~40GB/s; one big DMA reaches ~166GB/s).
    xp_d = nc.dram_tensor("xp", [128, NB * 8 * 512], BF16,
                          kind="ExternalInput")
    # weights are pre-shuffled on the host into the SBUF layout
    # [128, k_chunk * cols] so each load is one contiguous-per-partition
    # DMA (the naive strided layout costs ~25us in 1KB descriptors).
    wq_d = nc.dram_tensor("wq", [128, 8 * EG], BF16, kind="ExternalInput")
    wk_d = nc.dram_tensor("wk", [128, 8 * EG], BF16, kind="ExternalInput")
    wv_d = nc.dram_tensor("wv", [128, 8 * EG], BF16, kind="ExternalInput")
    wo_d = nc.dram_tensor("wo", [128, NJ * D], BF16, kind="ExternalInput")
    bq_d = nc.dram_tensor("bq", [128, NJ], F32, kind="ExternalInput")
    y_d = nc.dram_tensor("y", [T, D], BF16, kind="ExternalOutput")
    # scratch for the softmax-denominator reshape bounce: recip on [1,512]
    # costs 3.3us on one DVE lane; bounced to [64,16] it costs ~0.2us.
    ls_d = nc.dram_tensor("l_scratch", [NJ, NB, 2, 512], F32)
    rs_d = nc.dram_tensor("r_scratch", [NJ, NB, 2, 512], F32)

    with tile.TileContext(nc) as tc:
        from contextlib import ExitStack
        with ExitStack() as octx:
            main = octx.enter_context(tc.tile_pool(name="main", bufs=1))

            # bf16 q/k: the S stationary gets Fast Weight Load (2x) and
            # evacuations run at DVE 16-bit rate; costs ~2.5e-3 extra rel
            # err (9.6e-3 total vs 2e-2 budget).
            qT = [main.tile([128, T], BF16, name=f"qT{j}", tag=f"qT{j}")
                  for j in range(NJ)]
            kTp = [main.tile([128, T], BF16, name=f"kT{j}", tag=f"kT{j}")
                   for j in range(NJ)]
            v_sb = [main.tile([128, HG * VS], BF16, name=f"v{t}", tag=f"v{t}")
                    for t in range(NTK)]
            o_sb = [main.tile([128, T], BF16, name=f"o{j}", tag=f"o{j}")
                    for j in range(NJ)]
            xp_t = main.tile([128, NB * 8 * 512], BF16, name="xp", tag="xp")

            def xsl(k, q, off0, off1):
                # x chunk k, query quarter q, token offsets within quarter
                base = q * 4096 + k * 512
                return xp_t[:, base + off0:base + off1]
            # weight k-chunk k lives at cols [k*EG, (k+1)*EG)
            wq_t = main.tile([128, 8 * EG], BF16, name="wq", tag="wq")
            wk_t = main.tile([128, 8 * EG], BF16, name="wk", tag="wk")
            wv_t = main.tile([128, 8 * EG], BF16, name="wv", tag="wv")
            # wo pair j at cols [j*D, (j+1)*D)
            wo_t = main.tile([128, NJ * D], BF16, name="wo", tag="wo")
            bq_t = main.tile([128, NJ], F32, name="bq", tag="bq")
            # ones rows for the final-tail partition-broadcast matmul
            # (rows 0 and 64 used; matmul base partitions must be 0/64)
            on_t = main.tile([128, 64], BF16, name="ones1", tag="ones1")
            nc.vector.memset(on_t[:], 1.0)

            # ---- input DMAs ----
            # Only sync/scalar/gpsimd queues may initiate DMAs. One big
            # contiguous DMA per tensor (or half), spread over the three
            # queues by deadline: wk/wq/x-q0 gate the first exp (~15us),
            # wv + x-q1..3 gate the first block's V projections, wo is
            # only needed by the C chunks (~75% in).
            # DMA transfers from different queue slots all stream
            # concurrently and split HBM bandwidth, so without gating the
            # ~6MB of non-critical input steals ~2/3 of the bandwidth from
            # the critical wave (x-q0/wk/wq) and delays the first exp by
            # ~10us. Tiny copy "gates" read wave-N data and write the
            # first columns of wave-N+1's destination, giving the next
            # DMA a WAW dependency on the previous wave's landing. Gate
            # engine choice matters: each gate blocks its engine's queue
            # until the previous wave lands, so late-dependency gates live
            # on the otherwise-idle gpsimd queue, not on DVE (evacuation
            # casts) or ACT (the exp stream).
            SY, SC, GP = nc.sync, nc.scalar, nc.gpsimd

            # wave 1: critical for the first exp. Halves (aligned with the
            # k=0-3 / 4-7 projection halves) let the first projection
            # matmuls start while the second half is still in flight —
            # but ONLY with a serialization gate between the halves:
            # ungated transfers all stream concurrently at a fair share
            # of HBM bandwidth, so nothing lands early.
            SY.dma_start(bq_t[:], bq_d.ap())
            SY.dma_start(xp_t[:, 0:2048], xp_d.ap()[:, 0:2048])
            SC.dma_start(wk_t[:, 0:4 * EG], wk_d.ap()[:, 0:4 * EG])
            GP.dma_start(wq_t[:, 0:4 * EG], wq_d.ap()[:, 0:4 * EG])
            nc.vector.tensor_copy(xp_t[:, 2048:2052], xp_t[:, 0:4])
            SY.dma_start(xp_t[:, 2048:4096], xp_d.ap()[:, 2048:4096])
            nc.scalar.copy(wk_t[:, 4 * EG:4 * EG + 4], wk_t[:, 0:4])
            SC.dma_start(wk_t[:, 4 * EG:8 * EG], wk_d.ap()[:, 4 * EG:8 * EG])
            nc.gpsimd.tensor_copy(wq_t[:, 4 * EG:4 * EG + 4],
                                  wq_t[:, 0:4])
            GP.dma_start(wq_t[:, 4 * EG:8 * EG], wq_d.ap()[:, 4 * EG:8 * EG])
            # wave 2: wv halves + x quarter 1 (first block's V projection)
            nc.scalar.copy(wv_t[:, 0:4], wk_t[:, 4 * EG:4 * EG + 4])
            SC.dma_start(wv_t[:, 0:4 * EG], wv_d.ap()[:, 0:4 * EG])
            nc.gpsimd.tensor_copy(wv_t[:, 4 * EG:4 * EG + 4],
                                  wq_t[:, 4 * EG:4 * EG + 4])
            GP.dma_start(wv_t[:, 4 * EG:8 * EG], wv_d.ap()[:, 4 * EG:8 * EG])
            nc.vector.tensor_copy(xp_t[:, 4096:4100], xp_t[:, 2048:2052])
            SY.dma_start(xp_t[:, 4096:8192], xp_d.ap()[:, 4096:8192])
            # wave 3+: x quarters 2-3 and wo, serialized on gpsimd
            nc.gpsimd.tensor_copy(xp_t[:, 8192:8196],
                                  wv_t[:, 4 * EG:4 * EG + 4])
            GP.dma_start(xp_t[:, 8192:12288], xp_d.ap()[:, 8192:12288])
            nc.gpsimd.tensor_copy(xp_t[:, 12288:12292], xp_t[:, 8192:8196])
            GP.dma_start(xp_t[:, 12288:16384], xp_d.ap()[:, 12288:16384])
            nc.gpsimd.tensor_copy(wo_t[:, 0:4], xp_t[:, 12288:12292])
            GP.dma_start(wo_t[:], wo_d.ap())

            # ones column of each v tile (col 64 of each 66-stride group;
            # col 65 is pad, set to 1.0 too so the tile is fully init'd)
            for t in range(NTK):
                vv = v_sb[t][:].rearrange("p (h c) -> p h c", c=VS)
                nc.vector.memset(vv[:, :, HD:HD + 2], 1.0)

            # ---- warm-up: preload exp table and keep the PE HAM warm
            # through the ~12us DMA window so the first projections run at
            # 2.4GHz instead of 1.2 (each cold 512-col matmul costs 630ns
            # instead of 216).
            with tc.tile_pool(name="wu", bufs=1) as wup, \
                 tc.tile_pool(name="wu_ps", bufs=1, space="PSUM") as wups:
                wu_t = wup.tile([128, 512], BF16, name="wu")
                nc.vector.memset(wu_t[:], 0.0)
                wu_e = wup.tile([128, 8], F32, name="wue")
                # Ln FIRST: forces the natural_log_exp_and_others table
                # set (which also contains exp) from the start, so the
                # final tail's Ln does not insert a ~2.7us table swap
                # into the middle of the exp stream.
                nc.scalar.activation(wu_e[:], wu_t[:, 0:8], AF.Ln)
                nc.scalar.activation(wu_e[:], wu_t[:, 0:8], AF.Exp)
                wu_ps = wups.tile([128, 512], F32, name="wups")
                for i in range(6):
                    nc.tensor.matmul(wu_ps[:], wu_t[:, 0:128], wu_t[:],
                                     start=True, stop=True,
                                     skip_group_check=True)

            with tc.tile_pool(name="sp", bufs=1, space="PSUM") as spool, \
                 tc.tile_pool(name="op", bufs=1, space="PSUM") as opool, \
                 tc.tile_pool(name="fp", bufs=1, space="PSUM") as fpool, \
                 tc.tile_pool(name="pts", bufs=1) as ptpool, \
                 tc.tile_pool(name="aux", bufs=1) as aux:

                # Filler chunks are split into 4-matmul halves so each
                # pull adds ~860ns of PE work instead of ~1730ns: an
                # 8-matmul chunk between two exp tiles oversubscribes the
                # PE and stalls the ACT exp stream by ~1.1us per block.
                open_ps = {}

                def proj_half(kind, j, q, half):
                    qsl = slice(q * 512, (q + 1) * 512)
                    key = (kind, j, q)
                    if half == 0:
                        ps = fpool.tile([128, 512], F32, name="fps",
                                        tag="fps", bufs=2)
                        open_ps[key] = ps
                    else:
                        ps = open_ps.pop(key)
                    wt = wk_t if kind == 'k' else wq_t
                    for k in range(4 * half, 4 * half + 4):
                        nc.tensor.matmul(
                            ps[:],
                            wt[:, k * EG + j * 128:k * EG + (j + 1) * 128],
                            xsl(k, q, 0, 512),
                            start=(k == 0), stop=(k == 7))
                    if half == 1:
                        if kind == 'k':
                            nc.vector.tensor_copy(kTp[j][:, qsl], ps[:])
                        else:
                            nc.vector.tensor_scalar_add(qT[j][:, qsl], ps[:],
                                                        bq_t[:, j:j + 1])

                def v_half(ti, half):
                    key = ('v', ti)
                    if half == 0:
                        ps = fpool.tile([128, 512], F32, name="fps",
                                        tag="fps", bufs=2)
                        open_ps[key] = ps
                    else:
                        ps = open_ps.pop(key)
                    off = (ti % 4) * 128
                    for k in range(4 * half, 4 * half + 4):
                        nc.tensor.matmul(
                            ps[:], xsl(k, ti // 4, off, off + 128),
                            wv_t[:, k * EG:(k + 1) * EG],
                            start=(k == 0), stop=(k == 7))
                    if half == 1:
                        vv = v_sb[ti][:].rearrange("p (h c) -> p h c", c=VS)
                        nc.vector.tensor_copy(
                            vv[:, :, 0:HD],
                            ps[:].rearrange("p (h c) -> p h c", c=HD))

                def proj_chunk(kind, j, q):
                    proj_half(kind, j, q, 0)
                    proj_half(kind, j, q, 1)

                def v_chunk(ti):
                    v_half(ti, 0)
                    v_half(ti, 1)

                def c_chunk(tt, half):
                    tsl = slice(tt * 128, (tt + 1) * 128)
                    nsl = slice(half * 512, (half + 1) * 512)
                    ps = fpool.tile([128, 512], F32, name="fps", tag="fps",
                                    bufs=2)
                    for j in range(NJ):
                        nc.tensor.matmul(ps[:], o_sb[j][:, tsl],
                                         wo_t[:, j * D + half * 512:
                                              j * D + (half + 1) * 512],
                                         start=(j == 0), stop=(j == NJ - 1))
                    yt = aux.tile([128, 512], BF16, name="yt", tag="yt",
                                  bufs=3)
                    nc.vector.tensor_copy(yt[:], ps[:])
                    nc.sync.dma_start(y_d.ap()[tsl, nsl], yt[:])

                # ---- startup: k/q for pair 0 quarter 0 only — these gate
                # the first exp. v chunks go through the filler queue so
                # they sit AFTER the first S pair in PE program order
                # (emitted before it, a v chunk waiting on the wv DMA
                # would stall the PE and delay the first exp by ~10us).
                proj_chunk('k', 0, 0)
                proj_chunk('q', 0, 0)

                # ---- filler queue (4-matmul pull units) ----
                # ORDER IS LOAD-BEARING: Tile deps are created at emission
                # time, so every producer must be pulled before its first
                # consumer is emitted. With the pull rates below (4/p in
                # block 0; 4,4,2.. in block 1; 2/p in block 2; then 1/p):
                # k0qX lands before S reads kT quarter X at p=2X, v(t)
                # before PV(t) at p=t/2+2 (v12-15 before the deferred tail
                # at next-block p1), qJ/kJ before pair J starts at nblk 4J.
                chunks = [
                    ('k', 0, 1), ('v', 0, 0), ('v', 1, 0),
                    ('k', 0, 2), ('v', 2, 0), ('v', 3, 0),
                    ('v', 4, 0), ('v', 5, 0), ('v', 6, 0), ('v', 7, 0),
                    ('k', 0, 3), ('v', 8, 0), ('v', 9, 0),
                    ('v', 10, 0), ('v', 11, 0),
                    ('q', 0, 1),
                    ('v', 12, 0), ('v', 13, 0), ('v', 14, 0), ('v', 15, 0),
                    ('q', 0, 2), ('q', 0, 3),
                ]
                for jf in (1, 2, 3):
                    chunks.extend([('k', jf, q) for q in range(4)])
                    chunks.extend([('q', jf, q) for q in range(4)])
                filler = []
                for kind, a, bb_ in chunks:
                    for half in range(2):
                        if kind == 'v':
                            filler.append((kind, a, half))
                        else:
                            filler.append((kind, a, bb_, half))
                fidx = [0]

                def pull_filler(n=1):
                    for _ in range(n):
                        if fidx[0] >= len(filler):
                            return
                        ent = filler[fidx[0]]
                        fidx[0] += 1
                        kind = ent[0]
                        if kind == 'v':
                            v_half(ent[1], ent[2])
                        elif kind == 'c':
                            c_chunk(ent[1], ent[2])
                        else:
                            proj_half(ent[0], ent[1], ent[2], ent[3])

                def emit_pv(j, tk, ps_o, pts):
                    for h in range(2):
                        g = 2 * j + h
                        vcol = slice(g * VS, g * VS + HD + 1)
                        nc.tensor.matmul(
                            ps_o[h][:],
                            v_sb[tk][:, vcol],
                            pts[tk][:, h * 512:(h + 1) * 512],
                            start=(tk == 0), stop=(tk == NTK - 1))

                done_b = [0] * NB

                def make_tail(j, b, ps_o, pts, fast=False):
                    bsl = slice(b * 512, (b + 1) * 512)

                    def tail():
                        for tk in range(NTK - 4, NTK):
                            emit_pv(j, tk, ps_o, pts)
                        # stage O+l out of psum promptly, then normalize.
                        stage = [aux.tile([65, 512], F32, name=f"st{h}",
                                          tag=f"st{h}", bufs=2)
                                 for h in range(2)]
                        for h in range(2):
                            nc.vector.tensor_copy(stage[h][:], ps_o[h][:])
                        if fast:
                            # final tail: the DRAM reshape bounce costs
                            # ~10us of fully-exposed serial DMA latency at
                            # the end of the kernel (and lets the PE HAM
                            # re-throttle, doubling the final C chunks'
                            # cost). Instead: one-op approx reciprocal on
                            # the [1,512] l row, bf16 ones-matmul
                            # partition-broadcast, and dummy matmuls to
                            # keep the PE HAM warm through the DVE chain.
                            # 1/l = exp(-ln(l)) on the (now idle) ACT
                            # engine, reading l straight from PSUM; Ln and
                            # Exp share the natural_log_exp_and_others
                            # table set, so no table swap is inserted.
                            lg = aux.tile([65, 512], F32, name="lg",
                                          tag="lg")
                            rcb = aux.tile([65, 512], BF16, name="rcb",
                                           tag="rcb")
                            for h in range(2):
                                nc.scalar.activation(
                                    lg[h * 64:h * 64 + 1, :],
                                    ps_o[h][64:65, :], AF.Ln)
                                nc.scalar.activation(
                                    rcb[h * 64:h * 64 + 1, :],
                                    lg[h * 64:h * 64 + 1, :],
                                    AF.Exp, scale=-1.0)
                            wk_ps = spool.tile([128, 1024], F32,
                                               name="ps_s", tag="ps_s",
                                               bufs=2)
                            for i in range(8):
                                nc.tensor.matmul(
                                    wk_ps[0:64, 0:512],
                                    v_sb[0][:, 0:64], qT[0][:, 0:512],
                                    start=True, stop=True,
                                    skip_group_check=True)
                            rb = fpool.tile([128, 512], F32, name="fps",
                                            tag="fps", bufs=2)
                            for h in range(2):
                                nc.tensor.matmul(
                                    rb[h * 64:(h + 1) * 64, :],
                                    on_t[h * 64:h * 64 + 1, :],
                                    rcb[h * 64:h * 64 + 1, :],
                                    start=True, stop=True)
                            for h in range(2):
                                nc.vector.tensor_mul(
                                    o_sb[j][h * 64:(h + 1) * 64, bsl],
                                    stage[h][0:64, :],
                                    rb[h * 64:(h + 1) * 64, :])
                        else:
                            # out-of-band via the DRAM reshape bounce:
                            # recip on [1,512] costs ~3.3us on one DVE
                            # lane; bounced to [64,16] it costs ~0.2us and
                            # the DMA latency hides under the exp stream.
                            for h in range(2):
                                nc.sync.dma_start(ls_d.ap()[j, b, h, :],
                                                  stage[h][64:65, :])
                            lr = aux.tile([64, 16], F32, name="lr",
                                          tag="lr", bufs=2)
                            nc.sync.dma_start(
                                lr[:],
                                ls_d.ap()[j, b].rearrange(
                                    "h (a c) -> a h c", c=8))
                            rr = aux.tile([64, 16], F32, name="rr",
                                          tag="rr", bufs=2)
                            nc.vector.reciprocal(rr[:], lr[:])
                            nc.sync.dma_start(
                                rs_d.ap()[j, b].rearrange(
                                    "h (a c) -> a h c", c=8), rr[:])
                            rbc = aux.tile([64, 1024], F32, name="rbc",
                                           tag="rbc", bufs=2)
                            nc.sync.dma_start(
                                rbc[:].rearrange("p (h c) -> p h c", c=512),
                                rs_d.ap()[j, b][None, :, :]
                                .broadcast_to((64, 2, 512)))
                            for h in range(2):
                                nc.vector.tensor_mul(
                                    o_sb[j][h * 64:(h + 1) * 64, bsl],
                                    stage[h][0:64, :],
                                    rbc[:, h * 512:(h + 1) * 512])
                        done_b[b] += 1
                        if done_b[b] == NJ:
                            filler.extend([('c', tt, half)
                                           for tt in range(b * 4, b * 4 + 4)
                                           for half in range(2)])
                    return tail

                # ---- attention blocks ----
                # Steady-state PE budget per superslot (2 exp tiles,
                # ~2214ns of ACT): 2 S pairs ~500ns + 4 PVs ~864ns + one
                # 4-matmul pull unit ~864ns = ~2228ns. Anything above one
                # pull unit per superslot stalls the exp stream, so the
                # block-0 bootstrap (v chunks + pair-0 projections are
                # force-fed there) splits its pulls around the two exps.
                # PVs run two superslots behind their exp so the next
                # block's S pairs are emitted before the previous block's
                # last PVs + tail (kills the ~1.1us block-boundary gap);
                # the tail itself runs at p==1 of the next block for the
                # same reason.
                pending_tail = None
                for j in range(NJ):
                    for bb in range(NB):
                        b = (bb + j) % NB
                        bsl = slice(b * 512, (b + 1) * 512)
                        nblk = j * NB + bb
                        ps_o = None
                        pts = [None] * NTK

                        for p in range(NTK // 2):
                            if nblk == 0:
                                rate = 4
                            elif nblk == 1:
                                rate = 4 if p < 2 else 2
                            elif nblk >= 13:
                                # C-chunk era: the previous block's tail
                                # (4 PVs + normalize) lands at p==1, so
                                # shift that slot's pull to p2/p3.
                                rate = 0 if p == 1 else (2 if p in (2, 3)
                                                         else 1)
                            else:
                                rate = 1
                            for u in range(2):
                                tk = 2 * p + u
                                ksl = slice(tk * 128, (tk + 1) * 128)
                                ps_s = spool.tile([128, 1024], F32,
                                                  name="ps_s", tag="ps_s",
                                                  bufs=2)
                                for h in range(2):
                                    hp = slice(h * 64, (h + 1) * 64)
                                    nc.tensor.matmul(
                                        ps_s[:, h * 512:(h + 1) * 512],
                                        kTp[j][hp, ksl], qT[j][hp, bsl],
                                        start=True, stop=True)
                                pts[tk] = ptpool.tile([128, 1024], BF16,
                                                      name="pt", tag="pt",
                                                      bufs=12)
                                nc.scalar.activation(pts[tk][:], ps_s[:],
                                                     AF.Exp)
                                if rate >= 2:
                                    pull_filler(rate // 2)
                            if p == 1 and pending_tail is not None:
                                pending_tail()
                                pending_tail = None
                            if rate == 1:
                                pull_filler(1)
                            if p == 1:
                                ps_o = [opool.tile([65, 512], F32,
                                                   name=f"po{h}",
                                                   tag=f"po{h}", bufs=1)
                                        for h in range(2)]
                            if p > 1:
                                emit_pv(j, 2 * p - 4, ps_o, pts)
                                emit_pv(j, 2 * p - 3, ps_o, pts)
                        pending_tail = make_tail(
                            j, b, ps_o, pts,
                            fast=(j == NJ - 1 and bb == NB - 1))

                pending_tail()
                pending_tail = None

                # ---- drain remaining filler (last C chunks) ----
                while fidx[0] < len(filler):
                    pull_filler()

    nc.compile()
    return nc


_NC = None
_last_in_maps = None


def kernel(x, x_mean, x_std, Wq, bq, Wk, bk, Wv, bv, Wo, bo):
    global _NC
    if _NC is None:
        _NC = _build()

    bf = ml_dtypes.bfloat16
    x = np.asarray(x, dtype=np.float32)
    x_std = np.asarray(x_std, dtype=np.float32)
    Wq = np.asarray(Wq, dtype=np.float32)
    Wk = np.asarray(Wk, dtype=np.float32)
    Wv = np.asarray(Wv, dtype=np.float32)
    Wo = np.asarray(Wo, dtype=np.float32)
    bq = np.asarray(bq, dtype=np.float32)
    bv = np.asarray(bv, dtype=np.float32)
    bo = np.asarray(bo, dtype=np.float32)

    in_maps = []
    for c in range(N_CORES):
        b, g = c // 2, c % 2
        s = np.float32(SCALE / float(x_std[b, 0, 0]))
        rows = slice(g * EG, (g + 1) * EG)
        def shuf(wt, nchunk, cols):
            # [nchunk*128, cols] -> [128, nchunk*cols] (k-chunk-major cols)
            return np.ascontiguousarray(
                wt.reshape(nchunk, 128, cols).transpose(1, 0, 2)
                .reshape(128, nchunk * cols))
        # x packed quarter-major: [128 dims, quarter, k-chunk, 512 tokens]
        xp = (x[b].T.reshape(8, 128, 4, 512).transpose(1, 2, 0, 3)
              .reshape(128, 4 * 8 * 512))
        in_maps.append({
            "xp": np.ascontiguousarray(xp).astype(bf),
            "wq": shuf((Wq[rows, :] * s).T, 8, EG).astype(bf),
            "wk": shuf(Wk[rows, :].T, 8, EG).astype(bf),
            "wv": shuf(Wv[rows, :].T, 8, EG).astype(bf),
            "wo": shuf(Wo[:, rows].T, NJ, D).astype(bf),
            "bq": np.ascontiguousarray((bq[rows] * s).reshape(NJ, 128).T),
        })

    global _last_in_maps
    _last_in_maps = in_maps
    res = run_bass_kernel_spmd(_NC, in_maps, list(range(N_CORES)))

    bias_term = (bo + bv @ Wo.T).astype(np.float32)   # [D]
    y = np.empty((B, T, D), dtype=np.float32)
    for b in range(B):
        y[b] = (res.results[2 * b]["y"].astype(np.float32)
                + res.results[2 * b + 1]["y"].astype(np.float32)
                + bias_term[None, :])
    return y



# revision 46
# speedup vs baseline: 1.0181x; 1.0181x over previous
"""De-stationary attention on 8 Trainium2 NeuronCores — ACT-bound pipeline.

Problem: y = softmax((x Wq^T + bq)(x Wk^T + bk)^T * scale / (tau*x_std)) (x Wv^T + bv) Wo^T + bo
Shapes: x [4, 2048, 1024], 16 heads of 64 dims, tau=1, delta=0.

Sharding: core c handles batch b = c//2, head group g = c%2 (8 heads).
s = SCALE/x_std[b] is folded into Wq/bq on the host. Host sums the two
head-group partial y's per batch and adds bo + bv @ Wo^T.

Core schedule (the scalar engine's exp stream is the bottleneck at
~1113 ns per [128,1024] tile x 256 = 285 us; everything else hides
under it):
  - x, Wq/Wk/Wv/Wo are bf16 (rel err ~7e-3, budget 2e-2); qT/kT/S
    stay fp32(r).
  - S^T tiles [128 keys, 512 q] computed per head-pair with K=64
    row-tiled matmuls (tile_position (0,0)/(64,0) auto-derived from
    base partitions) -> both heads' S matmuls run concurrently.
  - exp: one ACTIVATE per tk over [128, 1024] psum (both heads),
    double-buffered; output P in bf16.
  - PV: per head, stationary [v | ones] [128, 65] bf16 -> psum
    [65, 512] accumulated over 16 tk; row 64 = softmax denominator l.
  - Normalize: stage O+l out of psum fast, then l -> DRAM -> [64,16]
    -> DVE reciprocal -> DRAM -> broadcast-read -> multiply (bf16).
  - Blocks j-outer (head pair), tq-block rotated b = (bb+j)%4 so the
    output projection for block b unlocks early; projections for
    pairs 1-3, V projection, and output-projection chunks are fed as
    PE "filler" inside the exp-paced loop. Each block's tail PVs and
    normalization are deferred into the next block so the next S pair
    (which feeds the exp stream) issues first.
Emission order = dependency order: Tile resolves deps at creation
time, so a producer emitted after its consumer becomes a WAR hazard.
PSUM: ps_s 2x[128,1024] (4 banks) + ps_o 2x[65,512] (2) + filler
2x[128,512] (2) = 8 banks.
"""

import os
import sys

for _p in ("/opt/trn_rl_repo", "/root/.axon_site/_ro/trn_rl_repo"):
    if os.path.isdir(_p) and _p not in sys.path:
        sys.path.insert(0, _p)

import numpy as np
import ml_dtypes

import concourse.bass as bass
import concourse.mybir as mybir
import concourse.tile as tile
from concourse import bacc
from concourse.bass_utils import run_bass_kernel_spmd

F32 = mybir.dt.float32
F32R = mybir.dt.float32r
BF16 = mybir.dt.bfloat16
AF = mybir.ActivationFunctionType

B, T, D, H = 4, 2048, 1024, 16
HD = D // H          # 64
SCALE = HD ** -0.5
HG = H // 2          # 8 heads per core
EG = HG * HD         # 512 projection dims per core
N_CORES = 8

NTK = T // 128       # 16 key tiles
NB = T // 512        # 4 query blocks of 512
NJ = HG // 2         # 4 head pairs per core
VS = HD + 2          # 66: v cols + ones col + pad (keeps bf16 4B align)


def _build():
    nc = bacc.Bacc("TRN2", target_bir_lowering=False, debug=False)

    # x is host-packed to [128, NB*8*512]: quarter-major, then k-chunk, so
    # each query-quarter is ONE contiguous 1MB DMA with 8KB rows (chunked
    # 1KB-row DMAs only reach ~40GB/s; one big DMA reaches ~166GB/s).
    xp_d = nc.dram_tensor("xp", [128, NB * 8 * 512], BF16,
                          kind="ExternalInput")
    # weights are pre-shuffled on the host into the SBUF layout
    # [128, k_chunk * cols] so each load is one contiguous-per-partition
    # DMA (the naive strided layout costs ~25us in 1KB descriptors).
    wq_d = nc.dram_tensor("wq", [128, 8 * EG], BF16, kind="ExternalInput")
    wk_d = nc.dram_tensor("wk", [128, 8 * EG], BF16, kind="ExternalInput")
    wv_d = nc.dram_tensor("wv", [128, 8 * EG], BF16, kind="ExternalInput")
    wo_d = nc.dram_tensor("wo", [128, NJ * D], BF16, kind="ExternalInput")
    bq_d = nc.dram_tensor("bq", [128, NJ], F32, kind="ExternalInput")
    y_d = nc.dram_tensor("y", [T, D], BF16, kind="ExternalOutput")
    # scratch for the softmax-denominator reshape bounce: recip on [1,512]
    # costs 3.3us on one DVE lane; bounced to [64,16] it costs ~0.2us.
    ls_d = nc.dram_tensor("l_scratch", [NJ, NB, 2, 512], F32)
    rs_d = nc.dram_tensor("r_scratch", [NJ, NB, 2, 512], F32)

    with tile.TileContext(nc) as tc:
        from contextlib import ExitStack
        with ExitStack() as octx:
            main = octx.enter_context(tc.tile_pool(name="main", bufs=1))

            # bf16 q/k: the S stationary gets Fast Weight Load (2x) and
            # evacuations run at DVE 16-bit rate; costs ~2.5e-3 extra rel
            # err (9.6e-3 total vs 2e-2 budget).
            qT = [main.tile([128, T], BF16, name=f"qT{j}", tag=f"qT{j}")
                  for j in range(NJ)]
            kTp = [main.tile([128, T], BF16, name=f"kT{j}", tag=f"kT{j}")
                   for j in range(NJ)]
            v_sb = [main.tile([128, HG * VS], BF16, name=f"v{t}", tag=f"v{t}")
                    for t in range(NTK)]
            o_sb = [main.tile([128, T], BF16, name=f"o{j}", tag=f"o{j}")
                    for j in range(NJ)]
            xp_t = main.tile([128, NB * 8 * 512], BF16, name="xp", tag="xp")

            def xsl(k, q, off0, off1):
                # x chunk k, query quarter q, token offsets within quarter
                base = q * 4096 + k * 512
                return xp_t[:, base + off0:base + off1]
            # weight k-chunk k lives at cols [k*EG, (k+1)*EG)
            wq_t = main.tile([128, 8 * EG], BF16, name="wq", tag="wq")
            wk_t = main.tile([128, 8 * EG], BF16, name="wk", tag="wk")
            wv_t = main.tile([128, 8 * EG], BF16, name="wv", tag="wv")
            # wo pair j at cols [j*D, (j+1)*D)
            wo_t = main.tile([128, NJ * D], BF16, name="wo", tag="wo")
            bq_t = main.tile([128, NJ], F32, name="bq", tag="bq")
            # ones rows for the final-tail partition-broadcast matmul
            # (rows 0 and 64 used; matmul base partitions must be 0/64)
            on_t = main.tile([128, 64], BF16, name="ones1", tag="ones1")
            nc.vector.memset(on_t[:], 1.0)

            # ---- input DMAs ----
            # Only sync/scalar/gpsimd queues may initiate DMAs. One big
            # contiguous DMA per tensor (or half), spread over the three
            # queues by deadline: wk/wq/x-q0 gate the first exp (~15us),
            # wv + x-q1..3 gate the first block's V projections, wo is
            # only needed by the C chunks (~75% in).
            # DMA transfers from different queue slots all stream
            # concurrently and split HBM bandwidth, so without gating the
            # ~6MB of non-critical input steals ~2/3 of the bandwidth from
            # the critical wave (x-q0/wk/wq) and delays the first exp by
            # ~10us. Tiny copy "gates" read wave-N data and write the
            # first columns of wave-N+1's destination, giving the next
            # DMA a WAW dependency on the previous wave's landing. Gate
            # engine choice matters: each gate blocks its engine's queue
            # until the previous wave lands, so late-dependency gates live
            # on the otherwise-idle gpsimd queue, not on DVE (evacuation
            # casts) or ACT (the exp stream).
            SY, SC, GP = nc.sync, nc.scalar, nc.gpsimd

            # wave 1: critical for the first exp. Halves (aligned with the
            # k=0-3 / 4-7 projection halves) let the first projection
            # matmuls start while the second half is still in flight —
            # but ONLY with a serialization gate between the halves:
            # ungated transfers all stream concurrently at a fair share
            # of HBM bandwidth, so nothing lands early.
            SY.dma_start(bq_t[:], bq_d.ap())
            SY.dma_start(xp_t[:, 0:2048], xp_d.ap()[:, 0:2048])
            SC.dma_start(wk_t[:, 0:4 * EG], wk_d.ap()[:, 0:4 * EG])
            GP.dma_start(wq_t[:, 0:4 * EG], wq_d.ap()[:, 0:4 * EG])
            nc.vector.tensor_copy(xp_t[:, 2048:2052], xp_t[:, 0:4])
            SY.dma_start(xp_t[:, 2048:4096], xp_d.ap()[:, 2048:4096])
            nc.vector.tensor_copy(wk_t[:, 4 * EG:4 * EG + 4], wk_t[:, 0:4])
            SC.dma_start(wk_t[:, 4 * EG:8 * EG], wk_d.ap()[:, 4 * EG:8 * EG])
            nc.gpsimd.tensor_copy(wq_t[:, 4 * EG:4 * EG + 4],
                                  wq_t[:, 0:4])
            GP.dma_start(wq_t[:, 4 * EG:8 * EG], wq_d.ap()[:, 4 * EG:8 * EG])
            # wave 2: wv halves + x quarter 1 (first block's V projection)
            nc.vector.tensor_copy(wv_t[:, 0:4], wk_t[:, 4 * EG:4 * EG + 4])
            SC.dma_start(wv_t[:, 0:4 * EG], wv_d.ap()[:, 0:4 * EG])
            nc.gpsimd.tensor_copy(wv_t[:, 4 * EG:4 * EG + 4],
                                  wq_t[:, 4 * EG:4 * EG + 4])
            GP.dma_start(wv_t[:, 4 * EG:8 * EG], wv_d.ap()[:, 4 * EG:8 * EG])
            nc.vector.tensor_copy(xp_t[:, 4096:4100], xp_t[:, 2048:2052])
            SY.dma_start(xp_t[:, 4096:8192], xp_d.ap()[:, 4096:8192])
            # wave 3+: x quarters 2-3 and wo, serialized on gpsimd
            nc.gpsimd.tensor_copy(xp_t[:, 8192:8196],
                                  wv_t[:, 4 * EG:4 * EG + 4])
            GP.dma_start(xp_t[:, 8192:12288], xp_d.ap()[:, 8192:12288])
            nc.gpsimd.tensor_copy(xp_t[:, 12288:12292], xp_t[:, 8192:8196])
            GP.dma_start(xp_t[:, 12288:16384], xp_d.ap()[:, 12288:16384])
            nc.gpsimd.tensor_copy(wo_t[:, 0:4], xp_t[:, 12288:12292])
            GP.dma_start(wo_t[:], wo_d.ap())

            # ones column of each v tile (col 64 of each 66-stride group;
            # col 65 is pad, set to 1.0 too so the tile is fully init'd)
            for t in range(NTK):
                vv = v_sb[t][:].rearrange("p (h c) -> p h c", c=VS)
                nc.vector.memset(vv[:, :, HD:HD + 2], 1.0)

            # ---- warm-up: preload exp table and keep the PE HAM warm
            # through the ~12us DMA window so the first projections run at
            # 2.4GHz instead of 1.2 (each cold 512-col matmul costs 630ns
            # instead of 216).
            with tc.tile_pool(name="wu", bufs=1) as wup, \
                 tc.tile_pool(name="wu_ps", bufs=1, space="PSUM") as wups:
                wu_t = wup.tile([128, 512], BF16, name="wu")
                nc.vector.memset(wu_t[:], 0.0)
                wu_e = wup.tile([128, 8], F32, name="wue")
                nc.scalar.activation(wu_e[:], wu_t[:, 0:8], AF.Exp)
                wu_ps = wups.tile([128, 512], F32, name="wups")
                for i in range(6):
                    nc.tensor.matmul(wu_ps[:], wu_t[:, 0:128], wu_t[:],
                                     start=True, stop=True,
                                     skip_group_check=True)

            with tc.tile_pool(name="sp", bufs=1, space="PSUM") as spool, \
                 tc.tile_pool(name="op", bufs=1, space="PSUM") as opool, \
                 tc.tile_pool(name="fp", bufs=1, space="PSUM") as fpool, \
                 tc.tile_pool(name="pts", bufs=1) as ptpool, \
                 tc.tile_pool(name="aux", bufs=1) as aux:

                # Filler chunks are split into 4-matmul halves so each
                # pull adds ~860ns of PE work instead of ~1730ns: an
                # 8-matmul chunk between two exp tiles oversubscribes the
                # PE and stalls the ACT exp stream by ~1.1us per block.
                open_ps = {}

                def proj_half(kind, j, q, half):
                    qsl = slice(q * 512, (q + 1) * 512)
                    key = (kind, j, q)
                    if half == 0:
                        ps = fpool.tile([128, 512], F32, name="fps",
                                        tag="fps", bufs=2)
                        open_ps[key] = ps
                    else:
                        ps = open_ps.pop(key)
                    wt = wk_t if kind == 'k' else wq_t
                    for k in range(4 * half, 4 * half + 4):
                        nc.tensor.matmul(
                            ps[:],
                            wt[:, k * EG + j * 128:k * EG + (j + 1) * 128],
                            xsl(k, q, 0, 512),
                            start=(k == 0), stop=(k == 7))
                    if half == 1:
                        if kind == 'k':
                            nc.vector.tensor_copy(kTp[j][:, qsl], ps[:])
                        else:
                            nc.vector.tensor_scalar_add(qT[j][:, qsl], ps[:],
                                                        bq_t[:, j:j + 1])

                def v_half(ti, half):
                    key = ('v', ti)
                    if half == 0:
                        ps = fpool.tile([128, 512], F32, name="fps",
                                        tag="fps", bufs=2)
                        open_ps[key] = ps
                    else:
                        ps = open_ps.pop(key)
                    off = (ti % 4) * 128
                    for k in range(4 * half, 4 * half + 4):
                        nc.tensor.matmul(
                            ps[:], xsl(k, ti // 4, off, off + 128),
                            wv_t[:, k * EG:(k + 1) * EG],
                            start=(k == 0), stop=(k == 7))
                    if half == 1:
                        vv = v_sb[ti][:].rearrange("p (h c) -> p h c", c=VS)
                        nc.vector.tensor_copy(
                            vv[:, :, 0:HD],
                            ps[:].rearrange("p (h c) -> p h c", c=HD))

                def proj_chunk(kind, j, q):
                    proj_half(kind, j, q, 0)
                    proj_half(kind, j, q, 1)

                def v_chunk(ti):
                    v_half(ti, 0)
                    v_half(ti, 1)

                # Output projection in two stages: a 3-matmul partial over
                # pairs 0-2 runs in the (otherwise empty) filler slack of
                # nblk 9-13, leaving only a cheap 1-matmul finalize after
                # pair 3's tail — the c-era was otherwise PE-oversubscribed
                # (4-matmul chunks that also stall on the normalize-bounce
                # latency when pulled too early).
                y_part = {}

                def pc_chunk(tt, half):
                    tsl = slice(tt * 128, (tt + 1) * 128)
                    ps = fpool.tile([128, 512], F32, name="fps", tag="fps",
                                    bufs=2)
                    for j in range(3):
                        nc.tensor.matmul(ps[:], o_sb[j][:, tsl],
                                         wo_t[:, j * D + half * 512:
                                              j * D + (half + 1) * 512],
                                         start=(j == 0), stop=(j == 2))
                    yp = aux.tile([128, 512], BF16, name=f"yp{tt}_{half}",
                                  tag=f"yp{tt}_{half}")
                    nc.vector.tensor_copy(yp[:], ps[:])
                    y_part[(tt, half)] = yp

                def fc_chunk(tt, half):
                    tsl = slice(tt * 128, (tt + 1) * 128)
                    nsl = slice(half * 512, (half + 1) * 512)
                    ps = fpool.tile([128, 512], F32, name="fps", tag="fps",
                                    bufs=2)
                    nc.tensor.matmul(ps[:], o_sb[3][:, tsl],
                                     wo_t[:, 3 * D + half * 512:
                                          3 * D + (half + 1) * 512],
                                     start=True, stop=True)
                    yt = aux.tile([128, 512], BF16, name="yt", tag="yt",
                                  bufs=3)
                    nc.vector.tensor_add(
                        yt[:], ps[:], y_part.pop((tt, half))[:])
                    nc.sync.dma_start(y_d.ap()[tsl, nsl], yt[:])

                # ---- startup: k/q for pair 0 quarter 0 only — these gate
                # the first exp. v chunks go through the filler queue so
                # they sit AFTER the first S pair in PE program order
                # (emitted before it, a v chunk waiting on the wv DMA
                # would stall the PE and delay the first exp by ~10us).
                proj_chunk('k', 0, 0)
                proj_chunk('q', 0, 0)

                # ---- filler queue (4-matmul pull units) ----
                # ORDER IS LOAD-BEARING: Tile deps are created at emission
                # time, so every producer must be pulled before its first
                # consumer is emitted. With the pull rates below (4/p in
                # block 0; 4,4,2.. in block 1; 2/p in block 2; then 1/p):
                # k0qX lands before S reads kT quarter X at p=2X, v(t)
                # before PV(t) at p=t/2+2 (v12-15 before the deferred tail
                # at next-block p1), qJ/kJ before pair J starts at nblk 4J.
                chunks = [
                    ('k', 0, 1), ('v', 0, 0), ('v', 1, 0),
                    ('k', 0, 2), ('v', 2, 0), ('v', 3, 0),
                    ('v', 4, 0), ('v', 5, 0), ('v', 6, 0), ('v', 7, 0),
                    ('k', 0, 3), ('v', 8, 0), ('v', 9, 0),
                    ('v', 10, 0), ('v', 11, 0),
                    ('q', 0, 1),
                    ('v', 12, 0), ('v', 13, 0), ('v', 14, 0), ('v', 15, 0),
                    ('q', 0, 2), ('q', 0, 3),
                ]
                for jf in (1, 2, 3):
                    chunks.extend([('k', jf, q) for q in range(4)])
                    chunks.extend([('q', jf, q) for q in range(4)])
                filler = []
                for kind, a, bb_ in chunks:
                    for half in range(2):
                        if kind == 'v':
                            filler.append((kind, a, half))
                        else:
                            filler.append((kind, a, bb_, half))
                fidx = [0]

                def pull_filler(n=1):
                    for _ in range(n):
                        if fidx[0] >= len(filler):
                            return
                        ent = filler[fidx[0]]
                        fidx[0] += 1
                        kind = ent[0]
                        if kind == 'v':
                            v_half(ent[1], ent[2])
                        elif kind == 'pc':
                            pc_chunk(ent[1], ent[2])
                        elif kind == 'fc':
                            fc_chunk(ent[1], ent[2])
                        else:
                            proj_half(ent[0], ent[1], ent[2], ent[3])

                def emit_pv(j, tk, ps_o, pts):
                    for h in range(2):
                        g = 2 * j + h
                        vcol = slice(g * VS, g * VS + HD + 1)
                        nc.tensor.matmul(
                            ps_o[h][:],
                            v_sb[tk][:, vcol],
                            pts[tk][:, h * 512:(h + 1) * 512],
                            start=(tk == 0), stop=(tk == NTK - 1))

                done_b = [0] * NB

                def make_tail(j, b, ps_o, pts, fast=False):
                    bsl = slice(b * 512, (b + 1) * 512)

                    def tail():
                        for tk in range(NTK - 4, NTK):
                            emit_pv(j, tk, ps_o, pts)
                        # stage O+l out of psum promptly, then normalize.
                        stage = [aux.tile([65, 512], F32, name=f"st{h}",
                                          tag=f"st{h}", bufs=2)
                                 for h in range(2)]
                        for h in range(2):
                            nc.vector.tensor_copy(stage[h][:], ps_o[h][:])
                        if fast:
                            # final tail: the DRAM reshape bounce costs
                            # ~10us of fully-exposed serial DMA latency at
                            # the end of the kernel (and lets the PE HAM
                            # re-throttle, doubling the final C chunks'
                            # cost). Instead: one-op approx reciprocal on
                            # the [1,512] l row, bf16 ones-matmul
                            # partition-broadcast, and dummy matmuls to
                            # keep the PE HAM warm through the DVE chain.
                            # 1/l = exp(-ln(l)) on the (now idle) ACT
                            # engine, reading l straight from PSUM; Ln and
                            # Exp share the natural_log_exp_and_others
                            # table set, so no table swap is inserted.
                            # Batched Ln,Ln then Exp,Exp: the table pass
                            # swaps sets per function change, so
                            # interleaving would cost 4 swaps instead of 2
                            # (both of which land after the exp stream).
                            lg = aux.tile([65, 512], F32, name="lg",
                                          tag="lg")
                            rcb = aux.tile([65, 512], BF16, name="rcb",
                                           tag="rcb")
                            for h in range(2):
                                nc.scalar.activation(
                                    lg[h * 64:h * 64 + 1, :],
                                    ps_o[h][64:65, :], AF.Ln)
                            for h in range(2):
                                nc.scalar.activation(
                                    rcb[h * 64:h * 64 + 1, :],
                                    lg[h * 64:h * 64 + 1, :],
                                    AF.Exp, scale=-1.0)
                            wk_ps = spool.tile([128, 1024], F32,
                                               name="ps_s", tag="ps_s",
                                               bufs=2)
                            for i in range(8):
                                nc.tensor.matmul(
                                    wk_ps[0:64, 0:512],
                                    v_sb[0][:, 0:64], qT[0][:, 0:512],
                                    start=True, stop=True,
                                    skip_group_check=True)
                            rb = fpool.tile([128, 512], F32, name="fps",
                                            tag="fps", bufs=2)
                            for h in range(2):
                                nc.tensor.matmul(
                                    rb[h * 64:(h + 1) * 64, :],
                                    on_t[h * 64:h * 64 + 1, :],
                                    rcb[h * 64:h * 64 + 1, :],
                                    start=True, stop=True)
                            for h in range(2):
                                nc.vector.tensor_mul(
                                    o_sb[j][h * 64:(h + 1) * 64, bsl],
                                    stage[h][0:64, :],
                                    rb[h * 64:(h + 1) * 64, :])
                        else:
                            # out-of-band via the DRAM reshape bounce:
                            # recip on [1,512] costs ~3.3us on one DVE
                            # lane; bounced to [64,16] it costs ~0.2us and
                            # the DMA latency hides under the exp stream.
                            for h in range(2):
                                nc.sync.dma_start(ls_d.ap()[j, b, h, :],
                                                  stage[h][64:65, :])
                            lr = aux.tile([64, 16], F32, name="lr",
                                          tag="lr", bufs=2)
                            nc.sync.dma_start(
                                lr[:],
                                ls_d.ap()[j, b].rearrange(
                                    "h (a c) -> a h c", c=8))
                            rr = aux.tile([64, 16], F32, name="rr",
                                          tag="rr", bufs=2)
                            nc.vector.reciprocal(rr[:], lr[:])
                            nc.sync.dma_start(
                                rs_d.ap()[j, b].rearrange(
                                    "h (a c) -> a h c", c=8), rr[:])
                            rbc = aux.tile([64, 1024], F32, name="rbc",
                                           tag="rbc", bufs=2)
                            nc.sync.dma_start(
                                rbc[:].rearrange("p (h c) -> p h c", c=512),
                                rs_d.ap()[j, b][None, :, :]
                                .broadcast_to((64, 2, 512)))
                            for h in range(2):
                                nc.vector.tensor_mul(
                                    o_sb[j][h * 64:(h + 1) * 64, bsl],
                                    stage[h][0:64, :],
                                    rbc[:, h * 512:(h + 1) * 512])
                        done_b[b] += 1
                        if done_b[b] == NJ - 1:
                            filler.extend([('pc', tt, half)
                                           for tt in range(b * 4, b * 4 + 4)
                                           for half in range(2)])
                        if done_b[b] == NJ:
                            filler.extend([('fc', tt, half)
                                           for tt in range(b * 4, b * 4 + 4)
                                           for half in range(2)])
                    return tail

                # ---- attention blocks ----
                # Steady-state PE budget per superslot (2 exp tiles,
                # ~2214ns of ACT): 2 S pairs ~500ns + 4 PVs ~864ns + one
                # 4-matmul pull unit ~864ns = ~2228ns. Anything above one
                # pull unit per superslot stalls the exp stream, so the
                # block-0 bootstrap (v chunks + pair-0 projections are
                # force-fed there) splits its pulls around the two exps.
                # PVs run two superslots behind their exp so the next
                # block's S pairs are emitted before the previous block's
                # last PVs + tail (kills the ~1.1us block-boundary gap);
                # the tail itself runs at p==1 of the next block for the
                # same reason.
                pending_tail = None
                for j in range(NJ):
                    for bb in range(NB):
                        b = (bb + j) % NB
                        bsl = slice(b * 512, (b + 1) * 512)
                        nblk = j * NB + bb
                        ps_o = None
                        pts = [None] * NTK

                        for p in range(NTK // 2):
                            if nblk == 0:
                                rate = 4
                            elif nblk == 1:
                                rate = 4 if p < 2 else 2
                            elif nblk >= 13:
                                # fc era: fc chunks stall on the previous
                                # block's normalize-bounce latency if
                                # pulled before ~p4, and the p1 slot
                                # already carries the deferred tail.
                                rate = 2 if p >= 4 else 0
                            else:
                                rate = 1
                            for u in range(2):
                                tk = 2 * p + u
                                ksl = slice(tk * 128, (tk + 1) * 128)
                                ps_s = spool.tile([128, 1024], F32,
                                                  name="ps_s", tag="ps_s",
                                                  bufs=2)
                                for h in range(2):
                                    hp = slice(h * 64, (h + 1) * 64)
                                    nc.tensor.matmul(
                                        ps_s[:, h * 512:(h + 1) * 512],
                                        kTp[j][hp, ksl], qT[j][hp, bsl],
                                        start=True, stop=True)
                                pts[tk] = ptpool.tile([128, 1024], BF16,
                                                      name="pt", tag="pt",
                                                      bufs=12)
                                nc.scalar.activation(pts[tk][:], ps_s[:],
                                                     AF.Exp)
                                if rate >= 2:
                                    pull_filler(rate // 2)
                            if p == 1 and pending_tail is not None:
                                pending_tail()
                                pending_tail = None
                            if rate == 1:
                                pull_filler(1)
                            if p == 1:
                                ps_o = [opool.tile([65, 512], F32,
                                                   name=f"po{h}",
                                                   tag=f"po{h}", bufs=1)
                                        for h in range(2)]
                            if p > 1:
                                emit_pv(j, 2 * p - 4, ps_o, pts)
                                emit_pv(j, 2 * p - 3, ps_o, pts)
                        pending_tail = make_tail(
                            j, b, ps_o, pts,
                            fast=(j == NJ - 1 and bb == NB - 1))

                pending_tail()
                pending_tail = None

                # ---- drain remaining filler (last C chunks) ----
                while fidx[0] < len(filler):
                    pull_filler()

    nc.compile()
    return nc


_NC = None
_last_in_maps = None


def kernel(x, x_mean, x_std, Wq, bq, Wk, bk, Wv, bv, Wo, bo):
    global _NC
    if _NC is None:
        _NC = _build()

    bf = ml_dtypes.bfloat16
    x = np.asarray(x, dtype=np.float32)
    x_std = np.asarray(x_std, dtype=np.float32)
    Wq = np.asarray(Wq, dtype=np.float32)
    Wk = np.asarray(Wk, dtype=np.float32)
    Wv = np.asarray(Wv, dtype=np.float32)
    Wo = np.asarray(Wo, dtype=np.float32)
    bq = np.asarray(bq, dtype=np.float32)
    bv = np.asarray(bv, dtype=np.float32)
    bo = np.asarray(bo, dtype=np.float32)

    in_maps = []
    for c in range(N_CORES):
        b, g = c // 2, c % 2
        s = np.float32(SCALE / float(x_std[b, 0, 0]))
        rows = slice(g * EG, (g + 1) * EG)
        def shuf(wt, nchunk, cols):
            # [nchunk*128, cols] -> [128, nchunk*cols] (k-chunk-major cols)
            return np.ascontiguousarray(
                wt.reshape(nchunk, 128, cols).transpose(1, 0, 2)
                .reshape(128, nchunk * cols))
        # x packed quarter-major: [128 dims, quarter, k-chunk, 512 tokens]
        xp = (x[b].T.reshape(8, 128, 4, 512).transpose(1, 2, 0, 3)
              .reshape(128, 4 * 8 * 512))
        in_maps.append({
            "xp": np.ascontiguousarray(xp).astype(bf),
            "wq": shuf((Wq[rows, :] * s).T, 8, EG).astype(bf),
            "wk": shuf(Wk[rows, :].T, 8, EG).astype(bf),
            "wv": shuf(Wv[rows, :].T, 8, EG).astype(bf),
            "wo": shuf(Wo[:, rows].T, NJ, D).astype(bf),
            "bq": np.ascontiguousarray((bq[rows] * s).reshape(NJ, 128).T),
        })

    global _last_in_maps
    _last_in_maps = in_maps
    res = run_bass_kernel_spmd(_NC, in_maps, list(range(N_CORES)))

    bias_term = (bo + bv @ Wo.T).astype(np.float32)   # [D]
    y = np.empty((B, T, D), dtype=np.float32)
    for b in range(B):
        y[b] = (res.results[2 * b]["y"].astype(np.float32)
                + res.results[2 * b + 1]["y"].astype(np.float32)
                + bias_term[None, :])
    return y

